# revision 15
# baseline (speedup 1.0000x reference)
"""ComplexDenseSO2 Trainium2 kernel.

Computes out = (X @ conj(B)^T * w) @ B for complex X [64, 32400],
B [2048, 32400], w [2048], given as separate re/im fp32 planes.

Strategy (tensor-parallel over D across 8 cores):
  - Fold w into the first-matmul operand on the host:
    M = diag(w) @ conj(B), so mm1 output IS Y = X @ M^T.
  - Pad D 32400 -> 32768; core c owns d-slice [c*4096, (c+1)*4096).
  - mm1 split in 2 k-chunks of 1024; per chunk accumulate over the 32
    d-tiles into 4 PSUM banks ([128j, 1024k] per plane).  Moving tiles
    are host-packed [128, 2048] (Mr|Mi) slabs -> 4KB DMA rows, DMAs
    alternate between the two HW DGE queues (sync + scalar).
  - Coeffs are transposed to k-major with the DMA XBAR transpose (no
    PE), combined re/im in free-dim slices, sent to a DRAM bounce and
    AllReduce'd per chunk so AR0 hides under mm1 chunk 1.
  - mm2: stationaries ytA=[Yr|Yi], ytB=[-Yi|Yr] sliced straight from
    the AR output; moving tiles are host-packed [128, 8192] (Br|Bi)
    slabs -> 16KB DMA rows, prefetched during the AR bubble.  The 8
    output PSUM banks reuse mm1's banks (pool tags), accumulating over
    all 16 k-blocks; epilogue descales and stores fp16.
  - fp16 operands use power-of-2 prescales (M*1024, B*256) to stay
    clear of fp16 subnormals; the epilogue descales by 2^-18.
"""

import sys

if "/opt/trn_rl_repo" not in sys.path:
    sys.path.insert(0, "/opt/trn_rl_repo")

import numpy as np

B_, K, D = 64, 2048, 32400
NCORES = 8
DP = 32768
DL = DP // NCORES  # 4096

SCALE_M = 1024.0
SCALE_B = 256.0

NKC = 2            # mm1 k-chunks / AR chunks
KCW = K // NKC     # 1024 k columns per chunk
NDT = DL // 128    # 32 d-tiles
NKB = K // 128     # 16 k-blocks (mm2 stationaries)
KBC = NKB // NKC   # k-blocks per AR chunk

_nc_cache = {}


def build_nc(n_cores=NCORES, k=K, dl=DL):
    import concourse.mybir as mybir
    from concourse import bacc
    import concourse.tile as tile
    from concourse.masks import make_identity

    fp = mybir.dt.float16
    f32 = mybir.dt.float32

    nc = bacc.Bacc(
        trn_type="TRN2",
        target_bir_lowering=False,
        debug=False,
        num_devices=n_cores,
    )
    # host-packed layouts (see _prep_in_maps)
    xt = nc.dram_tensor("xt", [128, dl], fp, kind="ExternalInput")
    mt = nc.dram_tensor("mt", [(NDT // 2) * 128, 2 * NKC * 2 * KCW], fp, kind="ExternalInput")
    bn = nc.dram_tensor("bn", [NKB * 128, 2 * dl], fp, kind="ExternalInput")
    out = nc.dram_tensor("out", [128, dl], fp, kind="ExternalOutput")

    with tile.TileContext(nc) as tc:
        with (
            tc.tile_pool(name="sb", bufs=2) as sb,
            tc.tile_pool(name="sbx", bufs=1) as sbx,
            tc.tile_pool(name="ps", bufs=1, space="PSUM") as ps,
            tc.tile_pool(name="dram", bufs=1, space="DRAM") as dram,
        ):
            ident = sbx.tile([128, 128], fp, tag="ident")
            make_identity(nc, ident)
            xts_all = sbx.tile([128, dl], fp, tag="xts_all", name="xts_all")
            nc.scalar.dma_start(out=xts_all, in_=xt.ap())
            xts = [xts_all[:, dt * 128 : (dt + 1) * 128] for dt in range(NDT)]

            arin = dram.tile([NKC * 128, KCW], fp, tag="arin", name="arin")
            dbar_in = dram.tile([1, 64], fp, tag="dbar_in", name="dbar_in")
            dbar_out = dram.tile(
                [1, 64], fp, tag="dbar_out", name="dbar_out", addr_space="Shared"
            )
            arout = dram.tile(
                [NKC * 128, KCW], fp, tag="arout", name="arout",
                addr_space="Shared",
            )

            # ---------------- mm1 (single pass over d, 8 PSUM banks) ----------------
            acc = [
                ps.tile([128, 512], f32, tag=f"pp{q}", name=f"pp{q}", bufs=2)
                for q in range(4)
            ] + [
                ps.tile([128, 512], f32, tag=f"pp{q}", name=f"pq{q}", bufs=2)
                for q in range(4)
            ]  # acc[0:2] re / acc[2:4] im of chunk0; acc[4:6] re / acc[6:8] im of chunk1
            STREAM_BUFS = 8
            NPAIR = NDT // 2
            for pair in range(NPAIR):
                eng = nc.sync if pair % 2 == 0 else nc.scalar
                mt_t = sb.tile(
                    [128, 8 * KCW], fp, tag="stream", name="mt", bufs=STREAM_BUFS
                )
                if pair == 0:
                    # split the first tile so dt=0 matmuls start ~5us earlier
                    nc.sync.dma_start(
                        out=mt_t[:, 0 : 4 * KCW],
                        in_=mt[0:128, 0 : 4 * KCW],
                    )
                    nc.sync.dma_start(
                        out=mt_t[:, 4 * KCW : 8 * KCW],
                        in_=mt[0:128, 4 * KCW : 8 * KCW],
                    )
                else:
                    eng.dma_start(
                        out=mt_t, in_=mt[pair * 128 : (pair + 1) * 128, :]
                    )
                last = pair == NPAIR - 1
                # on the last pair run chunk-0 columns first so its PSUM
                # banks stop (and evacuate) a few us earlier
                qorder = [0, 1, 2, 3, 4, 5, 6, 7]
                if last:
                    for q in qorder[:4]:
                        for half in range(2):
                            dt = 2 * pair + half
                            nc.tensor.matmul(
                                acc[q],
                                lhsT=xts[dt],
                                rhs=mt_t[
                                    :,
                                    half * 4 * KCW + q * 512 : half * 4 * KCW
                                    + (q + 1) * 512,
                                ],
                                start=False,
                                stop=dt == NDT - 1,
                            )
                    for q in qorder[4:]:
                        for half in range(2):
                            dt = 2 * pair + half
                            nc.tensor.matmul(
                                acc[q],
                                lhsT=xts[dt],
                                rhs=mt_t[
                                    :,
                                    half * 4 * KCW + q * 512 : half * 4 * KCW
                                    + (q + 1) * 512,
                                ],
                                start=False,
                                stop=dt == NDT - 1,
                            )
                else:
                    for half in range(2):
                        dt = 2 * pair + half
                        st = dt == 0
                        for q in qorder:
                            nc.tensor.matmul(
                                acc[q],
                                lhsT=xts[dt],
                                rhs=mt_t[
                                    :,
                                    half * 4 * KCW + q * 512 : half * 4 * KCW
                                    + (q + 1) * 512,
                                ],
                                start=st,
                                stop=False,
                            )
            a_rs, a_is = [], []
            for kc in range(NKC):
                a4 = acc[4 * kc : 4 * kc + 4]
                # evacuate PSUM -> fp16 SBUF (cast); frees all banks fast
                a_r = sb.tile([128, KCW], fp, tag=f"a_r{kc}", name=f"a_r{kc}")
                nc.vector.tensor_copy(a_r[:, 0:512], a4[0])
                nc.vector.tensor_copy(a_r[:, 512:1024], a4[1])
                a_i = sb.tile([128, KCW], fp, tag=f"a_i{kc}", name=f"a_i{kc}")
                nc.vector.tensor_copy(a_i[:, 0:512], a4[2])
                nc.vector.tensor_copy(a_i[:, 512:1024], a4[3])
                a_rs.append(a_r)
                a_is.append(a_i)
            for kc in range(NKC):
                a_r, a_i = a_rs[kc], a_is[kc]
                # k-major via PE transposes into fp16 PSUM banks
                tp_r = ps.tile([128, KCW], fp, tag="pp0", name=f"tp_r{kc}", bufs=2)
                tp_i = ps.tile([128, KCW], fp, tag="pp1", name=f"tp_i{kc}", bufs=2)
                for t in range(KCW // 128):
                    ts = slice(t * 128, (t + 1) * 128)
                    nc.tensor.transpose(tp_r[:, ts], a_r[:, ts], ident)
                    nc.tensor.transpose(tp_i[:, ts], a_i[:, ts], ident)
                # combine in k-major: j is now the free dim.  DVE may read
                # only one PSUM operand, so evacuate tp_r first.
                cc_r = sb.tile([128, KCW], fp, tag=f"cc_r{kc}", name=f"cc_r{kc}")
                nc.vector.tensor_copy(cc_r, tp_r)
                # Yr = re(X@Mr^T) - im(X@Mi^T); Yi = im(X@Mr^T) + re(X@Mi^T)
                c_t = sb.tile([128, KCW], fp, tag=f"c_t{kc}", name=f"c_t{kc}")
                c3 = c_t.rearrange("p (t j) -> p t j", j=128)
                r3 = cc_r.rearrange("p (t j) -> p t j", j=128)
                i3 = tp_i.rearrange("p (t j) -> p t j", j=128)
                nc.vector.tensor_sub(c3[:, :, 0:64], r3[:, :, 0:64], i3[:, :, 64:128])
                nc.vector.tensor_add(c3[:, :, 64:128], r3[:, :, 64:128], i3[:, :, 0:64])
                nc.sync.dma_start(out=arin[kc * 128 : (kc + 1) * 128, :], in_=c_t)

            # rendezvous: a tiny collective issued after the arin writes on the
            # same queue.  Its completion implies every core finished writing
            # arin, so the real AllReduce below never reads a half-written
            # remote buffer (observed race: cc_trigger can overlap the tail of
            # another core's arin DMA).
            dbar_sb = sb.tile([1, 64], fp, tag="dbar_sb", name="dbar_sb")
            nc.vector.tensor_copy(dbar_sb, c_t[0:1, 0:64])
            nc.sync.dma_start(out=dbar_in, in_=dbar_sb)
            nc.gpsimd.collective_compute(
                "AllReduce",
                mybir.AluOpType.add,
                ins=[dbar_in.opt()],
                outs=[dbar_out.opt()],
                replica_groups=[list(range(n_cores))],
            )

            # ---------------- AllReduce ----------------
            nc.gpsimd.collective_compute(
                "AllReduce",
                mybir.AluOpType.add,
                ins=[arin.opt()],
                outs=[arout.opt()],
                replica_groups=[list(range(n_cores))],
            )

            # ---------------- mm2 ----------------
            po = [
                ps.tile(
                    [128, 512], f32, tag=f"pp{h % 4}", name=f"po{h}", bufs=2
                )
                for h in range(8)
            ]
            bn_tiles = []
            for kb in range(NKB):
                eng = nc.sync if kb % 2 == 0 else nc.scalar
                bn_t = sb.tile(
                    [128, 2 * dl], fp, tag="stream", name="bn", bufs=STREAM_BUFS
                )
                eng.dma_start(out=bn_t, in_=bn[kb * 128 : (kb + 1) * 128, :])
                bn_tiles.append(bn_t)
                if kb == 5:
                    # AR-dependent reads go after the prefetch window so the
                    # DMA queues keep streaming during the collective.
                    for kc in range(NKC):
                        ytA = sbx.tile(
                            [128, KCW], fp, tag=f"ytA{kc}", name=f"ytA{kc}"
                        )
                        nc.sync.dma_start(
                            out=ytA, in_=arout[kc * 128 : (kc + 1) * 128, :]
                        )
                        ytB = sbx.tile(
                            [128, KCW], fp, tag=f"ytB{kc}", name=f"ytB{kc}"
                        )
                        for b in range(KBC):
                            s = b * 128
                            nc.vector.tensor_scalar_mul(
                                ytB[:, s : s + 64],
                                ytA[:, s + 64 : s + 128],
                                -1.0,
                            )
                            nc.vector.tensor_copy(
                                ytB[:, s + 64 : s + 128], ytA[:, s : s + 64]
                            )
                        if kc == 0:
                            ytA0, ytB0 = ytA, ytB
                        else:
                            ytA1, ytB1 = ytA, ytB
            yts = [(ytA0, ytB0), (ytA1, ytB1)]
            for kc in range(NKC):
                ytA, ytB = yts[kc]
                for b in range(KBC):
                    kb = kc * KBC + b
                    bn_eff = bn_tiles[kb]
                    st, sp = kb == 0, kb == NKB - 1
                    s = b * 128
                    for h in range(8):
                        nc.tensor.matmul(
                            po[h],
                            lhsT=ytA[:, s : s + 128],
                            rhs=bn_eff[:, h * 512 : (h + 1) * 512],
                            start=st,
                            stop=False,
                        )
                    for h in range(8):
                        nc.tensor.matmul(
                            po[h],
                            lhsT=ytB[:, s : s + 128],
                            rhs=bn_eff[:, dl + h * 512 : dl + (h + 1) * 512],
                            start=False,
                            stop=sp,
                        )
            for h in range(8):
                o_t = sb.tile([128, 512], fp, tag="o_t", name="o_t", bufs=4)
                nc.vector.tensor_scalar_mul(o_t, po[h], 1.0 / (SCALE_M * SCALE_B))
                eng = nc.sync if h % 2 == 0 else nc.scalar
                eng.dma_start(out=out[:, h * 512 : (h + 1) * 512], in_=o_t)

    nc.compile()
    return nc


def _get_nc(n_cores=NCORES, k=K, dl=DL):
    key = (n_cores, k, dl)
    if key not in _nc_cache:
        _nc_cache[key] = build_nc(n_cores, k, dl)
    return _nc_cache[key]


def _prep_in_maps(X_re, X_im, bases_re, bases_im, weight_re, weight_im):
    cdt = np.float16
    f32 = np.float32
    X_re = np.asarray(X_re, f32)
    X_im = np.asarray(X_im, f32)
    bases_re = np.asarray(bases_re, f32)
    bases_im = np.asarray(bases_im, f32)
    wr = np.asarray(weight_re, f32)[:, None]
    wi = np.asarray(weight_im, f32)[:, None]

    # M = diag(w) @ conj(B): Mr = wr*Br + wi*Bi ; Mi = wi*Br - wr*Bi
    mr = (wr * bases_re + wi * bases_im) * np.float32(SCALE_M)
    mi = (wi * bases_re - wr * bases_im) * np.float32(SCALE_M)
    bsr = (bases_re * np.float32(SCALE_B)).astype(cdt)
    bsi = (bases_im * np.float32(SCALE_B)).astype(cdt)
    mr = mr.astype(cdt)
    mi = mi.astype(cdt)

    in_maps = []
    for c in range(NCORES):
        lo = c * DL
        hi = min((c + 1) * DL, D)
        n = hi - lo

        # xt[p, dt*128 + j] = Xstack^T[dt*128+p, j], j: 0:64 re, 64:128 im
        xtd = np.zeros((DL, 128), cdt)
        xtd[:n, 0:64] = X_re[:, lo:hi].T.astype(cdt)
        xtd[:n, 64:128] = X_im[:, lo:hi].T.astype(cdt)
        xt = (
            xtd.reshape(NDT, 128, 128).transpose(1, 0, 2).reshape(128, DL)
        )

        # mt[(kc*NDT+dt)*128 + p, :] = [MrT[d, kc-chunk] | MiT[d, kc-chunk]]
        mrT = np.zeros((DL, K), cdt)
        miT = np.zeros((DL, K), cdt)
        mrT[:n, :] = mr[:, lo:hi].T
        miT[:n, :] = mi[:, lo:hi].T
        mt = np.empty((NDT, 128, NKC, 2 * KCW), cdt)
        r4 = mrT.reshape(NDT, 128, NKC, KCW)
        i4 = miT.reshape(NDT, 128, NKC, KCW)
        for kc in range(NKC):
            mt[:, :, kc, 0:KCW] = r4[:, :, kc, :]
            mt[:, :, kc, KCW:] = i4[:, :, kc, :]
        # group dt-pairs: row block holds dt=2p then dt=2p+1 side by side
        mt = (
            mt.reshape(NDT // 2, 2, 128, NKC * 2 * KCW)
            .transpose(0, 2, 1, 3)
            .reshape((NDT // 2) * 128, 2 * NKC * 2 * KCW)
        )

        # bn[kb*128 + p, :] = [Br[k, d-shard] | Bi[k, d-shard]]
        bnd = np.zeros((K, 2 * DL), cdt)
        bnd[:, 0:n] = bsr[:, lo:hi]
        bnd[:, DL : DL + n] = bsi[:, lo:hi]

        in_maps.append({"xt": xt, "mt": mt, "bn": bnd})
    return in_maps


def run(inputs, trace=False, trace_kwargs=None):
    """Returns (full complex64 output [64, 32400], BassKernelResults)."""
    from concourse.bass_utils import run_bass_kernel_spmd

    in_maps = _prep_in_maps(**inputs)
    nc = _get_nc()
    res = run_bass_kernel_spmd(
        nc,
        in_maps,
        core_ids=list(range(NCORES)),
        trace=trace,
        **(trace_kwargs or {}),
    )
    parts = []
    inv = np.float32(1.0)
    for c in range(NCORES):
        o = res.results[c]["out"].astype(np.float32) * inv
        parts.append(o[0:64, :] + 1j * o[64:128, :].astype(np.complex64))
    full = np.concatenate(parts, axis=1)[:, :D].astype(np.complex64)
    return full, res


def kernel(**inputs) -> np.ndarray:
    out, _ = run(inputs, trace=False)
    return out


# revision 16
# speedup vs baseline: 1.0403x; 1.0403x over previous
"""ComplexDenseSO2 Trainium2 kernel.

Computes out = (X @ conj(B)^T * w) @ B for complex X [64, 32400],
B [2048, 32400], w [2048], given as separate re/im fp32 planes.

Strategy (tensor-parallel over D across 8 cores):
  - Fold w into the first-matmul operand on the host:
    M = diag(w) @ conj(B), so mm1 output IS Y = X @ M^T.
  - Pad D 32400 -> 32768; core c owns d-slice [c*4096, (c+1)*4096).
  - mm1 split in 2 k-chunks of 1024; per chunk accumulate over the 32
    d-tiles into 4 PSUM banks ([128j, 1024k] per plane).  Moving tiles
    are host-packed [128, 2048] (Mr|Mi) slabs -> 4KB DMA rows, DMAs
    alternate between the two HW DGE queues (sync + scalar).
  - Coeffs are transposed to k-major with the DMA XBAR transpose (no
    PE), combined re/im in free-dim slices, sent to a DRAM bounce and
    AllReduce'd per chunk so AR0 hides under mm1 chunk 1.
  - mm2: stationaries ytA=[Yr|Yi], ytB=[-Yi|Yr] sliced straight from
    the AR output; moving tiles are host-packed [128, 8192] (Br|Bi)
    slabs -> 16KB DMA rows, prefetched during the AR bubble.  The 8
    output PSUM banks reuse mm1's banks (pool tags), accumulating over
    all 16 k-blocks; epilogue descales and stores fp16.
  - fp16 operands use power-of-2 prescales (M*1024, B*256) to stay
    clear of fp16 subnormals; the epilogue descales by 2^-18.
"""

import sys

if "/opt/trn_rl_repo" not in sys.path:
    sys.path.insert(0, "/opt/trn_rl_repo")

import numpy as np

B_, K, D = 64, 2048, 32400
NCORES = 8
DP = 32768
DL = DP // NCORES  # 4096

SCALE_M = 1024.0
SCALE_B = 256.0

NKC = 2            # mm1 k-chunks / AR chunks
KCW = K // NKC     # 1024 k columns per chunk
NDT = DL // 128    # 32 d-tiles
NKB = K // 128     # 16 k-blocks (mm2 stationaries)
KBC = NKB // NKC   # k-blocks per AR chunk

_nc_cache = {}


def build_nc(n_cores=NCORES, k=K, dl=DL):
    import concourse.mybir as mybir
    from concourse import bacc
    import concourse.tile as tile
    from concourse.masks import make_identity

    fp = mybir.dt.bfloat16
    f32 = mybir.dt.float32

    nc = bacc.Bacc(
        trn_type="TRN2",
        target_bir_lowering=False,
        debug=False,
        num_devices=n_cores,
    )
    # host-packed layouts (see _prep_in_maps)
    xt = nc.dram_tensor("xt", [128, dl], fp, kind="ExternalInput")
    mt = nc.dram_tensor("mt", [(NDT // 2) * 128, 2 * NKC * 2 * KCW], fp, kind="ExternalInput")
    bn = nc.dram_tensor("bn", [NKB * 128, 2 * dl], fp, kind="ExternalInput")
    out = nc.dram_tensor("out", [128, dl], fp, kind="ExternalOutput")

    with tile.TileContext(nc) as tc:
        with (
            tc.tile_pool(name="sb", bufs=2) as sb,
            tc.tile_pool(name="sbx", bufs=1) as sbx,
            tc.tile_pool(name="ps", bufs=1, space="PSUM") as ps,
            tc.tile_pool(name="dram", bufs=1, space="DRAM") as dram,
        ):
            ident = sbx.tile([128, 128], fp, tag="ident")
            make_identity(nc, ident)
            xts_all = sbx.tile([128, dl], fp, tag="xts_all", name="xts_all")
            nc.scalar.dma_start(out=xts_all, in_=xt.ap())
            xts = [xts_all[:, dt * 128 : (dt + 1) * 128] for dt in range(NDT)]

            arin = dram.tile([NKC * 128, KCW], fp, tag="arin", name="arin")
            dbar_in = dram.tile([1, 64], fp, tag="dbar_in", name="dbar_in")
            dbar_out = dram.tile(
                [1, 64], fp, tag="dbar_out", name="dbar_out", addr_space="Shared"
            )
            arout = dram.tile(
                [NKC * 128, KCW], fp, tag="arout", name="arout",
                addr_space="Shared",
            )

            # ---------------- mm1 (single pass over d, 8 PSUM banks) ----------------
            acc = [
                ps.tile([128, 512], f32, tag=f"pp{q}", name=f"pp{q}", bufs=2)
                for q in range(4)
            ] + [
                ps.tile([128, 512], f32, tag=f"pp{q}", name=f"pq{q}", bufs=2)
                for q in range(4)
            ]  # acc[0:2] re / acc[2:4] im of chunk0; acc[4:6] re / acc[6:8] im of chunk1
            STREAM_BUFS = 8
            NPAIR = NDT // 2
            for pair in range(NPAIR):
                eng = nc.sync if pair % 2 == 0 else nc.scalar
                mt_t = sb.tile(
                    [128, 8 * KCW], fp, tag="stream", name="mt", bufs=STREAM_BUFS
                )
                if pair == 0:
                    # split the first tile so dt=0 matmuls start ~5us earlier
                    nc.sync.dma_start(
                        out=mt_t[:, 0 : 4 * KCW],
                        in_=mt[0:128, 0 : 4 * KCW],
                    )
                    nc.sync.dma_start(
                        out=mt_t[:, 4 * KCW : 8 * KCW],
                        in_=mt[0:128, 4 * KCW : 8 * KCW],
                    )
                else:
                    eng.dma_start(
                        out=mt_t, in_=mt[pair * 128 : (pair + 1) * 128, :]
                    )
                last = pair == NPAIR - 1
                # on the last pair run chunk-0 columns first so its PSUM
                # banks stop (and evacuate) a few us earlier
                qorder = [0, 1, 2, 3, 4, 5, 6, 7]
                if last:
                    for q in qorder[:4]:
                        for half in range(2):
                            dt = 2 * pair + half
                            nc.tensor.matmul(
                                acc[q],
                                lhsT=xts[dt],
                                rhs=mt_t[
                                    :,
                                    half * 4 * KCW + q * 512 : half * 4 * KCW
                                    + (q + 1) * 512,
                                ],
                                start=False,
                                stop=dt == NDT - 1,
                            )
                    for q in qorder[4:]:
                        for half in range(2):
                            dt = 2 * pair + half
                            nc.tensor.matmul(
                                acc[q],
                                lhsT=xts[dt],
                                rhs=mt_t[
                                    :,
                                    half * 4 * KCW + q * 512 : half * 4 * KCW
                                    + (q + 1) * 512,
                                ],
                                start=False,
                                stop=dt == NDT - 1,
                            )
                else:
                    for half in range(2):
                        dt = 2 * pair + half
                        st = dt == 0
                        for q in qorder:
                            nc.tensor.matmul(
                                acc[q],
                                lhsT=xts[dt],
                                rhs=mt_t[
                                    :,
                                    half * 4 * KCW + q * 512 : half * 4 * KCW
                                    + (q + 1) * 512,
                                ],
                                start=st,
                                stop=False,
                            )
            a_rs, a_is = [], []
            for kc in range(NKC):
                a4 = acc[4 * kc : 4 * kc + 4]
                # evacuate PSUM -> fp16 SBUF (cast); frees all banks fast
                a_r = sb.tile([128, KCW], fp, tag=f"a_r{kc}", name=f"a_r{kc}")
                nc.vector.tensor_copy(a_r[:, 0:512], a4[0])
                nc.vector.tensor_copy(a_r[:, 512:1024], a4[1])
                a_i = sb.tile([128, KCW], fp, tag=f"a_i{kc}", name=f"a_i{kc}")
                nc.vector.tensor_copy(a_i[:, 0:512], a4[2])
                nc.vector.tensor_copy(a_i[:, 512:1024], a4[3])
                a_rs.append(a_r)
                a_is.append(a_i)
            for kc in range(NKC):
                a_r, a_i = a_rs[kc], a_is[kc]
                # k-major via PE transposes into fp16 PSUM banks
                tp_r = ps.tile([128, KCW], fp, tag="pp0", name=f"tp_r{kc}", bufs=2)
                tp_i = ps.tile([128, KCW], fp, tag="pp1", name=f"tp_i{kc}", bufs=2)
                for t in range(KCW // 128):
                    ts = slice(t * 128, (t + 1) * 128)
                    nc.tensor.transpose(tp_r[:, ts], a_r[:, ts], ident)
                    nc.tensor.transpose(tp_i[:, ts], a_i[:, ts], ident)
                # combine in k-major: j is now the free dim.  DVE may read
                # only one PSUM operand, so evacuate tp_r first.
                cc_r = sb.tile([128, KCW], fp, tag=f"cc_r{kc}", name=f"cc_r{kc}")
                nc.vector.tensor_copy(cc_r, tp_r)
                # Yr = re(X@Mr^T) - im(X@Mi^T); Yi = im(X@Mr^T) + re(X@Mi^T)
                c_t = sb.tile([128, KCW], fp, tag=f"c_t{kc}", name=f"c_t{kc}")
                c3 = c_t.rearrange("p (t j) -> p t j", j=128)
                r3 = cc_r.rearrange("p (t j) -> p t j", j=128)
                i3 = tp_i.rearrange("p (t j) -> p t j", j=128)
                nc.vector.tensor_sub(c3[:, :, 0:64], r3[:, :, 0:64], i3[:, :, 64:128])
                nc.vector.tensor_add(c3[:, :, 64:128], r3[:, :, 64:128], i3[:, :, 0:64])
                nc.sync.dma_start(out=arin[kc * 128 : (kc + 1) * 128, :], in_=c_t)

            # rendezvous: a tiny collective issued after the arin writes on the
            # same queue.  Its completion implies every core finished writing
            # arin, so the real AllReduce below never reads a half-written
            # remote buffer (observed race: cc_trigger can overlap the tail of
            # another core's arin DMA).
            dbar_sb = sb.tile([1, 64], fp, tag="dbar_sb", name="dbar_sb")
            nc.vector.tensor_copy(dbar_sb, c_t[0:1, 0:64])
            nc.sync.dma_start(out=dbar_in, in_=dbar_sb)
            nc.gpsimd.collective_compute(
                "AllReduce",
                mybir.AluOpType.add,
                ins=[dbar_in.opt()],
                outs=[dbar_out.opt()],
                replica_groups=[list(range(n_cores))],
            )

            # ---------------- AllReduce ----------------
            nc.gpsimd.collective_compute(
                "AllReduce",
                mybir.AluOpType.add,
                ins=[arin.opt()],
                outs=[arout.opt()],
                replica_groups=[list(range(n_cores))],
            )

            # ---------------- mm2 ----------------
            po = [
                ps.tile(
                    [128, 512], f32, tag=f"pp{h % 4}", name=f"po{h}", bufs=2
                )
                for h in range(8)
            ]
            bn_tiles = []
            for kb in range(NKB):
                eng = nc.sync if kb % 2 == 0 else nc.scalar
                bn_t = sb.tile(
                    [128, 2 * dl], fp, tag="stream", name="bn", bufs=STREAM_BUFS
                )
                eng.dma_start(out=bn_t, in_=bn[kb * 128 : (kb + 1) * 128, :])
                bn_tiles.append(bn_t)
                if kb == 5:
                    # AR-dependent reads go after the prefetch window so the
                    # DMA queues keep streaming during the collective.
                    for kc in range(NKC):
                        ytA = sbx.tile(
                            [128, KCW], fp, tag=f"ytA{kc}", name=f"ytA{kc}"
                        )
                        nc.sync.dma_start(
                            out=ytA, in_=arout[kc * 128 : (kc + 1) * 128, :]
                        )
                        ytB = sbx.tile(
                            [128, KCW], fp, tag=f"ytB{kc}", name=f"ytB{kc}"
                        )
                        for b in range(KBC):
                            s = b * 128
                            nc.vector.tensor_scalar_mul(
                                ytB[:, s : s + 64],
                                ytA[:, s + 64 : s + 128],
                                -1.0,
                            )
                            nc.vector.tensor_copy(
                                ytB[:, s + 64 : s + 128], ytA[:, s : s + 64]
                            )
                        if kc == 0:
                            ytA0, ytB0 = ytA, ytB
                        else:
                            ytA1, ytB1 = ytA, ytB
            yts = [(ytA0, ytB0), (ytA1, ytB1)]
            for kc in range(NKC):
                ytA, ytB = yts[kc]
                for b in range(KBC):
                    kb = kc * KBC + b
                    bn_eff = bn_tiles[kb]
                    st, sp = kb == 0, kb == NKB - 1
                    s = b * 128
                    for h in range(8):
                        nc.tensor.matmul(
                            po[h],
                            lhsT=ytA[:, s : s + 128],
                            rhs=bn_eff[:, h * 512 : (h + 1) * 512],
                            start=st,
                            stop=False,
                        )
                    for h in range(8):
                        nc.tensor.matmul(
                            po[h],
                            lhsT=ytB[:, s : s + 128],
                            rhs=bn_eff[:, dl + h * 512 : dl + (h + 1) * 512],
                            start=False,
                            stop=sp,
                        )
            for h in range(8):
                o_t = sb.tile([128, 512], fp, tag="o_t", name="o_t", bufs=4)
                nc.vector.tensor_scalar_mul(o_t, po[h], 1.0 / (SCALE_M * SCALE_B))
                eng = nc.sync if h % 2 == 0 else nc.scalar
                eng.dma_start(out=out[:, h * 512 : (h + 1) * 512], in_=o_t)

    nc.compile()
    return nc


def _get_nc(n_cores=NCORES, k=K, dl=DL):
    key = (n_cores, k, dl)
    if key not in _nc_cache:
        _nc_cache[key] = build_nc(n_cores, k, dl)
    return _nc_cache[key]


def _prep_in_maps(X_re, X_im, bases_re, bases_im, weight_re, weight_im):
    import ml_dtypes

    cdt = ml_dtypes.bfloat16
    f32 = np.float32
    X_re = np.asarray(X_re, f32)
    X_im = np.asarray(X_im, f32)
    bases_re = np.asarray(bases_re, f32)
    bases_im = np.asarray(bases_im, f32)
    wr = np.asarray(weight_re, f32)[:, None]
    wi = np.asarray(weight_im, f32)[:, None]

    # M = diag(w) @ conj(B): Mr = wr*Br + wi*Bi ; Mi = wi*Br - wr*Bi
    mr = (wr * bases_re + wi * bases_im) * np.float32(SCALE_M)
    mi = (wi * bases_re - wr * bases_im) * np.float32(SCALE_M)
    bsr = (bases_re * np.float32(SCALE_B)).astype(cdt)
    bsi = (bases_im * np.float32(SCALE_B)).astype(cdt)
    mr = mr.astype(cdt)
    mi = mi.astype(cdt)

    in_maps = []
    for c in range(NCORES):
        lo = c * DL
        hi = min((c + 1) * DL, D)
        n = hi - lo

        # xt[p, dt*128 + j] = Xstack^T[dt*128+p, j], j: 0:64 re, 64:128 im
        xtd = np.zeros((DL, 128), cdt)
        xtd[:n, 0:64] = X_re[:, lo:hi].T.astype(cdt)
        xtd[:n, 64:128] = X_im[:, lo:hi].T.astype(cdt)
        xt = (
            xtd.reshape(NDT, 128, 128).transpose(1, 0, 2).reshape(128, DL)
        )

        # mt[(kc*NDT+dt)*128 + p, :] = [MrT[d, kc-chunk] | MiT[d, kc-chunk]]
        mrT = np.zeros((DL, K), cdt)
        miT = np.zeros((DL, K), cdt)
        mrT[:n, :] = mr[:, lo:hi].T
        miT[:n, :] = mi[:, lo:hi].T
        mt = np.empty((NDT, 128, NKC, 2 * KCW), cdt)
        r4 = mrT.reshape(NDT, 128, NKC, KCW)
        i4 = miT.reshape(NDT, 128, NKC, KCW)
        for kc in range(NKC):
            mt[:, :, kc, 0:KCW] = r4[:, :, kc, :]
            mt[:, :, kc, KCW:] = i4[:, :, kc, :]
        # group dt-pairs: row block holds dt=2p then dt=2p+1 side by side
        mt = (
            mt.reshape(NDT // 2, 2, 128, NKC * 2 * KCW)
            .transpose(0, 2, 1, 3)
            .reshape((NDT // 2) * 128, 2 * NKC * 2 * KCW)
        )

        # bn[kb*128 + p, :] = [Br[k, d-shard] | Bi[k, d-shard]]
        bnd = np.zeros((K, 2 * DL), cdt)
        bnd[:, 0:n] = bsr[:, lo:hi]
        bnd[:, DL : DL + n] = bsi[:, lo:hi]

        in_maps.append({"xt": xt, "mt": mt, "bn": bnd})
    return in_maps


def run(inputs, trace=False, trace_kwargs=None):
    """Returns (full complex64 output [64, 32400], BassKernelResults)."""
    from concourse.bass_utils import run_bass_kernel_spmd

    in_maps = _prep_in_maps(**inputs)
    nc = _get_nc()
    res = run_bass_kernel_spmd(
        nc,
        in_maps,
        core_ids=list(range(NCORES)),
        trace=trace,
        **(trace_kwargs or {}),
    )
    parts = []
    inv = np.float32(1.0)
    for c in range(NCORES):
        o = res.results[c]["out"].astype(np.float32) * inv
        parts.append(o[0:64, :] + 1j * o[64:128, :].astype(np.complex64))
    full = np.concatenate(parts, axis=1)[:, :D].astype(np.complex64)
    return full, res


def kernel(**inputs) -> np.ndarray:
    out, _ = run(inputs, trace=False)
    return out
